# revision 59
# baseline (speedup 1.0000x reference)
"""nn_Attention_18700287607351 — GQA attention (RMSNorm + RoPE) on 8 TRN2 cores.

Sharding (per the hint): 8 shards = (batch in {0,1}) x (4 KV-head groups),
each shard owns 2 KV heads + their 8 query heads, with Wq/Wk/Wv rows and Wo
columns split by head group. Each core computes a partial [T, D] output
(row-parallel Wo); the host sums the 4 partials per batch.

Device kernel (per core, mixed bf16 / fp8-DoubleRow):
  - Precision split: the error-sensitive work — everything consumed by
    low-key-count (early) query rows — stays bf16: token-tile 0 of the
    Q/K/V projections, every diagonal 128-chunk of scores/exp/attnV, and
    the whole O-GEMM. Everything consumed only by >=512-key rows runs as
    fp8e4m3 DoubleRow matmuls (0.5 PE cycles/output column, 2x contraction
    per pass): projection tiles 1-3, off-diagonal scores, paired attnV.
    Attention averaging washes fp8 noise by ~1.65/sqrt(n_keys), so late
    rows stay ~4e-3 accurate while early rows take the bf16 path.
  - All Wq/Wk/Wv are host-scaled 16x so their fp8 copies avoid e4m3
    subnormals; RMSNorm cancels the scale for q/k, and the 16x on v rides
    through attnV + O-GEMM and is divided out in the host gather.
  - exp is computed with a folded bias of -2 (softmax-invariant) so fp8
    P' = exp(s/8 - 2) <= e^6 = 403 < 448 can never overflow e4m3.
  - Scores q8/k8 live in a (32-partition, pair) DoubleRow layout packed 4
    heads per 128 partitions (PE quadrant tile_position); the DR pair dim
    contracts dims d = 32*i + p. attnV DoubleRow instead pairs two
    128-token chunks (the pair dim = chunk parity), which is exactly the
    [128, 2, 512] layout the paired exp already produces, so one DR matmul
    consumes both chunks of a pair.
  - Otherwise the structure matches the bf16 baseline: feature-major
    qT/kT, host-permuted head dims (evens|odds) making RoPE
    partition-aligned, per-head RMS via block-ones matmuls, transposed
    scores with causal block-skipping, ones-column denominator in attnV,
    O-GEMM software-pipelined into the next tile's attention heads.
"""

import numpy as np
import ml_dtypes

import concourse.bass as bass
import concourse.bacc as bacc
import concourse.mybir as mybir
import concourse.tile as tile
from concourse.bass import ts
from concourse.bass_utils import run_bass_kernel_spmd

BF16 = ml_dtypes.bfloat16
E4M3 = ml_dtypes.float8_e4m3fn

B, T, D = 2, 2048, 2048
H, HKV, HD = 32, 8, 64
THETA = 3.0
EPS = 1e-6
SCALE = HD ** -0.5
N_CORES = 8
GROUPS = N_CORES // B          # 4 head-groups per batch
KVH = HKV // GROUPS            # 2 kv heads per core
QH = KVH * (H // HKV)          # 8 q heads per core
CQ = QH * HD                   # 512 q cols per core
CKV = KVH * HD                 # 128 kv cols per core
P = 128
WS = 16.0                      # host weight scale (fp8 subnormal avoidance)
EBIAS = -2.0                   # exp bias, softmax-invariant, fp8 overflow-proof

F32 = mybir.dt.float32
BF = mybir.dt.bfloat16
F8 = mybir.dt.float8e4
DR = mybir.MatmulPerfMode.DoubleRow


def _emit_iteration(nc, tc, ap, t_len, taps=None):
    """Emit one full per-core attention iteration into the TileContext."""
    Tl = t_len
    DC = D // P              # d chunks (16)
    DC8 = D // 256           # DoubleRow d chunk pairs (8)
    QC = CQ // P             # q col chunks (4)
    NT = Tl // 512           # 512-wide t tiles
    TC = Tl // P             # 128-wide t chunks

    import contextlib
    ctx = contextlib.ExitStack()
    const = ctx.enter_context(tc.tile_pool(name="const", bufs=1))
    projctx = contextlib.ExitStack()
    proj = projctx.enter_context(tc.tile_pool(name="proj", bufs=1))
    workp = projctx.enter_context(tc.tile_pool(name="workp", bufs=2))
    psum1 = projctx.enter_context(tc.tile_pool(name="psum1", space="PSUM", bufs=1))
    work = workp   # phase-1 alias; re-pointed to the attention pool below
    psum = psum1   # phase-1 alias

    # ---------------- persistent loads ----------------
    ntab = ap["tabs"].shape[1]
    tabs_early = const.tile([P, ntab, Tl], BF, name="tabs")
    xT_sb = proj.tile([P, DC, 512], BF)             # tokens 0..511 only (bf16 path)
    x8_sb = proj.tile([P, DC8, 2, Tl - 512], F8)    # tokens 512+ (DR path)
    wqT_sb = proj.tile([P, DC, CQ], BF)
    wkT_sb = proj.tile([P, DC, CKV], BF)
    wvT_sb = proj.tile([P, DC, CKV], BF)
    wq8_sb = proj.tile([P, DC8, 2, CQ], F8)
    wk8_sb = proj.tile([P, DC8, 2, CKV], F8)
    wv8_sb = proj.tile([P, DC8, 2, CKV], F8)
    woT_sb = const.tile([P, QC, D], BF)
    # one DMA per tensor; [(c p) n -> p c n] so each partition reads DC
    # contiguous segments. Order: K/V weights first, then x so the K-GEMM
    # accumulation can chase the loads; Wo (needed last) at the end.
    nc.sync.dma_start(out=wkT_sb, in_=ap["wkT"])
    xTview = ap["xT"].rearrange("(c p) t -> p c t", p=P)
    for c4 in range(4):
        nc.sync.dma_start(out=xT_sb[:, c4 * 4:(c4 + 1) * 4], in_=xTview[:, c4 * 4:(c4 + 1) * 4])
    nc.sync.dma_start(out=wvT_sb, in_=ap["wvT"])
    nc.sync.dma_start(out=wqT_sb, in_=ap["wqT"].rearrange("(c p) n -> p c n", p=P))
    nc.sync.dma_start(out=tabs_early, in_=ap["tabs"])
    nc.sync.dma_start(out=wk8_sb, in_=ap["wk8"])
    nc.sync.dma_start(out=wv8_sb, in_=ap["wv8"])
    nc.sync.dma_start(out=wq8_sb, in_=ap["wq8"].rearrange("(c i p) n -> p c i n", p=P, i=2))
    x8view = ap["x8"].rearrange("(c i p) t -> p c i t", p=P, i=2)
    for c2 in range(4):
        nc.sync.dma_start(out=x8_sb[:, c2 * 2:(c2 + 1) * 2], in_=x8view[:, c2 * 2:(c2 + 1) * 2])
    nc.sync.dma_start(out=woT_sb, in_=ap["woT"].rearrange("(c p) n -> p c n", p=P))
    tabs = tabs_early

    eps_col = const.tile([P, 1], F32)
    nc.vector.memset(eps_col, EPS)
    ebias_col = const.tile([P, 1], F32)
    nc.vector.memset(ebias_col, EBIAS)
    perm_sb = const.tile([P, P], BF)
    nc.sync.dma_start(out=perm_sb, in_=ap["perm"])
    trimask = const.tile([P, P], BF)
    nc.sync.dma_start(out=trimask, in_=ap["trimask"])
    trimask2 = const.tile([P, 2 * P], BF)   # [zeros | tri] for merged diag pairs
    nc.vector.memset(trimask2[:, 0:P], 0.0)
    nc.sync.dma_start(out=trimask2[:, P:2 * P], in_=ap["trimask"])
    blk128 = const.tile([P, P], BF)         # block-ones: per-head sum + broadcast
    nc.vector.memset(blk128, 0.0)
    nc.vector.memset(blk128[0:64, 0:64], 1.0)
    nc.vector.memset(blk128[64:128, 64:128], 1.0)

    # persistent activations
    q8flat = proj.tile([P, QC, Tl], F8)        # fp8 q staging (pre-shuffle)
    k8flat = proj.tile([P, Tl], F8)            # fp8 k staging (pre-shuffle)
    qT_sb = const.tile([P, QC, Tl], BF)        # feature-major q, rms+rope'd
    kT_sb = const.tile([P, Tl], BF)            # feature-major k [kv0 | kv1]
    kT_sw = const.tile([P, Tl], BF)            # swapped copy [kv1 | kv0]
    v_tok = const.tile([P, TC, KVH, HD + 1], BF)  # token-major v + ones col
    oT_sb = const.tile([P, QC, Tl], BF)        # feature-major attn out
    # fp8 DoubleRow copies: q8/k8 pack head quadrants on 32-partition bases;
    # pair dim contracts d = 32*i + p. v8 pairs two 128-token chunks.
    q8_sb = const.tile([P, 2, QH // 2, Tl], F8)   # [64b+p, i, h//2, t], b=h%2
    k8_sb = const.tile([P, 2, KVH, Tl], F8)       # kv replicated at bases {0,64}
    # dual-fp8 ldweights needs the pair stride 64-aligned: inner width 128
    v8_tok = const.tile([P, TC // 2, KVH, 2, 2 * HD], F8)

    def rms_rope(src_psum, dst, nt, tab0):
        """src_psum [128, 512] f32 (2 heads feature-major) -> dst bf16 slice.

        Row layout per head h in {0,1}: rows h*64..h*64+31 = even dims (x1),
        h*64+32..h*64+63 = odd dims (x2). RoPE = xs*TCC + swap32(xs)*TSS with
        the sign and norm_w baked into the host-built tables, so every DVE op
        is partition-aligned (walrus requires samePartitionsAll).
        """
        sq = work.tile([P, 512], BF, tag="sq", bufs=2)
        nc.scalar.activation(sq, src_psum, mybir.ActivationFunctionType.Square)
        # per-head sum over 64-partition groups, broadcast to all 64 rows
        # by the block-ones matmul itself -> [128, 512]
        SS = psum.tile([P, 512], F32, tag="ms" if int(__import__("os").environ.get("KJ0", "1")) < 2 else "sbig", bufs=2)
        nc.tensor.matmul(SS, blk128, sq, start=True, stop=True)
        rt = work.tile([P, 512], F32, tag="rr", bufs=2)
        nc.scalar.activation(rt, SS, mybir.ActivationFunctionType.Sqrt,
                             scale=1.0 / HD, bias=eps_col)
        with nc.allow_low_precision(reason="rsqrt broadcast is plenty for a 2e-2 gate"):
            nc.vector.reciprocal(rt, rt)
        xs = work.tile([P, 512], BF, tag="xs", bufs=2)
        nc.vector.tensor_mul(xs, src_psum, rt)
        # rope partner (row m ^ 32) via permutation matmul
        xw_ps = psum.tile([P, 512], F32, tag="sbig", bufs=2)
        nc.tensor.matmul(xw_ps, perm_sb, xs, start=True, stop=True)
        m1 = work.tile([P, 512], BF, tag="m1", bufs=2)
        nc.vector.tensor_mul(m1, xs, tabs[:, tab0 + 0, ts(nt, 512)])
        xw = work.tile([P, 512], BF, tag="xw", bufs=2)
        nc.scalar.copy(xw, xw_ps)
        m2 = work.tile([P, 512], BF, tag="m2", bufs=2)
        nc.vector.tensor_mul(m2, xw, tabs[:, tab0 + 1, ts(nt, 512)])
        nc.vector.tensor_add(dst, m1, m2)

    def proj_matmuls(dst_ps, wT, w8, mcols, nt):
        """Accumulate x @ W.T for token tile nt into dst_ps [128, 512].

        Tile 0 runs bf16 (early rows need it); tiles 1+ run fp8 DoubleRow.
        mcols = (col0, ncols) slice of the weight output dim.
        """
        c0, ncol = mcols
        if nt == 0:
            for c in range(DC):
                nc.tensor.matmul(dst_ps, wT[:, c, c0:c0 + ncol],
                                 xT_sb[:, c, :],
                                 start=(c == 0), stop=(c == DC - 1))
        else:
            for c in range(DC8):
                nc.tensor.matmul(dst_ps, w8[:, c, :, c0:c0 + ncol],
                                 x8_sb[:, c, :, ts(nt - 1, 512)],
                                 start=(c == 0), stop=(c == DC8 - 1),
                                 perf_mode=DR)

    def to_fp8(src_bf, dst_f8):
        """bf16 -> fp8 dtype conversion on the (mostly idle) Pool engine."""
        nc.gpsimd.tensor_copy(out=dst_f8, in_=src_bf)

    # ---------------- K / V / Q, software-pipelined ----------------
    # Emit unit u+1's GEMM chain BEFORE unit u's rms/post ops so the
    # in-order PE queue never stalls head-of-line behind ACT/DVE work;
    # continuous PE bursts also keep the tensor engine at full p-state.
    def post_K(ps, nt):
        rms_rope(ps, kT_sb[:, ts(nt, 512)], nt, ntab - 2)
        # swapped copy so every q-head finds its kv head at its own base
        # partition (matmul requires lhsT/rhs partition bases to match)
        nc.sync.dma_start(out=kT_sw[0:64, ts(nt, 512)], in_=kT_sb[64:128, ts(nt, 512)])
        nc.sync.dma_start(out=kT_sw[64:128, ts(nt, 512)], in_=kT_sb[0:64, ts(nt, 512)])
        to_fp8(kT_sb[:, ts(nt, 512)], k8flat[:, ts(nt, 512)])

    def post_V(ps, nt):
        vT_tmp = work.tile([P, 512], BF, tag="vT", bufs=2)
        nc.scalar.copy(vT_tmp, ps)
        for i4 in range(4):
            tchunk = nt * 4 + i4
            # XBAR transpose needs a contiguous destination; stage then
            # copy into the strided [.., kv, 0:64] layout on gpsimd.
            vstg = work.tile([P, P], BF, tag="vstg")
            nc.sync.dma_start(out=vstg, in_=vT_tmp[:, ts(i4, P)], transpose=True)
            nc.gpsimd.tensor_copy(
                out=v_tok[:, tchunk, :, 0:HD],
                in_=vstg.rearrange("p (h c) -> p h c", h=KVH))
            nc.gpsimd.tensor_copy(
                out=v8_tok[:, tchunk // 2, :, tchunk % 2, 0:HD],
                in_=vstg.rearrange("p (h c) -> p h c", h=KVH))

    def post_Q(ps, nt, mc):
        rms_rope(ps, qT_sb[:, mc, ts(nt, 512)], nt, 0)
        if nt > 0:
            # fp8 copies only feed off-diagonal scores (query tiles 1+)
            to_fp8(qT_sb[:, mc, ts(nt, 512)], q8flat[:, mc, ts(nt, 512)])

    units = []
    for nt in range(NT):
        units.append(("K", nt, None))
        units.append(("V", nt, None))
        units.extend(("Q", nt, mc) for mc in range(QC))
    inflight = []

    def drain_one():
        pkind, pnt, pmc, pps = inflight.pop(0)
        if pkind == "K":
            post_K(pps, pnt)
        elif pkind == "V":
            post_V(pps, pnt)
        else:
            post_Q(pps, pnt, pmc)

    for uidx, (kind, nt, mc) in enumerate(units):
        ps = psum.tile([P, 512], F32, tag="opsum" if uidx % 2 == 0 else "qkv",
                       bufs=2, name="ps")
        if kind == "K":
            proj_matmuls(ps, wkT_sb, wk8_sb, (0, CKV), nt)
        elif kind == "V":
            proj_matmuls(ps, wvT_sb, wv8_sb, (0, CKV), nt)
        else:
            proj_matmuls(ps, wqT_sb, wq8_sb, (mc * P, P), nt)
        inflight.append((kind, nt, mc, ps))
        import os as _os
        if len(inflight) > int(_os.environ.get('KRA', '1')):
            drain_one()
    while inflight:
        drain_one()
    nc.vector.memset(v_tok[:, :, :, HD], 1.0)
    nc.vector.memset(v8_tok[:, :, :, :, HD], 1.0)


    # ---------------- attention + output projection, tile-major ----------------
    # j outer / heads inner so the O-GEMM for tile j's rows can start right
    # after its 8 heads finish, filling PE idle under the ACT-bound exp phase.
    import os
    phase = os.environ.get("KPHASE", "all")
    attn_tiles = list(range(NT)) if phase in ("all", "attn") else []

    odue = []

    ob_cur = [None]
    o_alt = [0]
    o_half = [None]

    def o_finish(tt, nd, o_out):
        for mc in (2, 3):
            nc.tensor.matmul(o_out, oT_sb[:, mc, ts(tt, P)],
                             woT_sb[:, mc, ts(nd, 512)],
                             start=False, stop=(mc == QC - 1))
        if nd == 0:
            ob_cur[0] = work.tile([P, D], BF, tag="ob", bufs=4, name="ob")
        nc.vector.tensor_copy(ob_cur[0][:, ts(nd, 512)], o_out)
        if nd == D // 512 - 1:
            nc.sync.dma_start(out=ap["outp"][ts(tt, P), :], in_=ob_cur[0])

    def o_unit(nmax=1, alt=False):
        # one [128-token, 512-outcol] O-GEMM unit: 4 PE matmuls + DVE copy;
        # the 4 units of a token chunk share one ob tile and one output DMA.
        # Mid-tile drips emit HALF units (2 matmuls) so the PE-queue detour
        # between score pairs stays under ACT's per-pair exp time; the
        # drain (alt=True) runs full units.
        for _ in range(nmax):
            if o_half[0] is not None:
                tt, nd, o_out = o_half[0]
                o_half[0] = None
                o_finish(tt, nd, o_out)
                continue
            if not odue:
                return
            tt, nd = odue.pop(0)
            o_alt[0] += 1
            tag = "qkv" if not alt or o_alt[0] % 2 else "opsum"
            o_out = psum.tile([P, 512], F32, tag=tag, bufs=2, name="o_out")
            for mc in (0, 1):
                nc.tensor.matmul(o_out, oT_sb[:, mc, ts(tt, P)],
                                 woT_sb[:, mc, ts(nd, 512)],
                                 start=(mc == 0), stop=False)
            if alt:
                o_finish(tt, nd, o_out)
            else:
                o_half[0] = (tt, nd, o_out)

    import os as _os2
    _kj = int(_os2.environ.get("KJ0", "1"))
    for jidx, j in enumerate(attn_tiles):
        if jidx == 1:
            # fp8 DoubleRow shuffles: one wide partition-shift DMA per
            # (head, i) — batched here so no waiting DMA ever blocks the
            # in-order SP queue mid-phase
            for kv in range(KVH):
                for i in range(2):
                    srcv = k8flat[64 * kv + 32 * i:64 * kv + 32 * i + 32, :]
                    for b in range(2):
                        nc.sync.dma_start(out=k8_sb[64 * b:64 * b + 32, i, kv, :],
                                          in_=srcv)
            for h8 in range(QH):
                mc8, half8 = h8 // 2, h8 % 2
                for i in range(2):
                    nc.sync.dma_start(
                        out=q8_sb[64 * half8:64 * half8 + 32, i, h8 // 2, 512:],
                        in_=q8flat[64 * half8 + 32 * i:64 * half8 + 32 * i + 32,
                                   mc8, 512:])
        if jidx == _kj:
            # release projection inputs + phase-1 work/psum; later tiles use
            # the attention pools (earlier tiles ran on phase-1 pools,
            # overlapped under the projection tail)
            projctx.close()
            work = ctx.enter_context(tc.tile_pool(name="worka", bufs=2))
            psum = ctx.enter_context(tc.tile_pool(name="psum2", space="PSUM", bufs=1))
        if phase == "all" and jidx > 0:
            jp = attn_tiles[jidx - 1]
            odue.extend((4 * jp + t4, nd) for t4 in range(4)
                        for nd in range(D // 512))
        for h in range(QH):
            # previous tile's output projection units drip in only after
            # each pair flush — never at a head/tile boundary, where the
            # next head's scores must reach ACT without a PE detour
            kv = h // (QH // KVH)
            mc, half = h // 2, h % 2
            q_rows = qT_sb[half * 64:(half + 1) * 64, mc, :]
            ksrc = kT_sb if kv == half else kT_sw
            k_rows = ksrc[half * 64:(half + 1) * 64, :]
            b64 = 64 * (h % 2)
            q8_rows = q8_sb[b64:b64 + 32, :, h // 2, :]
            k8_rows = k8_sb[b64:b64 + 32, :, kv, :]
            o_ps = psum.tile([HD + 1, 512], F32, tag="opsum", bufs=2)
            nchunks = 4 * (j + 1)

            # full (below-diagonal) chunks in PAIRS: fp8 DoubleRow scores
            # (2x), one exp instruction over [128, 2, 512] writing fp8
            # (amortizes the ACT PSUM access penalty), then ONE DoubleRow
            # attnV matmul consuming both chunks (pair dim = chunk parity).
            # Scores run one pair AHEAD of exp/attnV so the PE queue never
            # stalls head-of-line behind an exp it just fed.
            def flush_full(pend):
                sp2, i = pend
                pt8 = work.tile([P, 2, 512], F8, tag="pt8", bufs=2 if jidx < _kj else 6)
                nc.scalar.activation(pt8, sp2, mybir.ActivationFunctionType.Exp,
                                     scale=SCALE, bias=ebias_col)
                nc.tensor.matmul(o_ps, v8_tok[:, i // 2, kv, :, 0:HD + 1], pt8,
                                 start=(i == 0), stop=False, perf_mode=DR)

            pend = None
            for i in range(0, 4 * j, 2):
                sp2 = psum.tile([P, 2, 512], F32, tag="sbig", bufs=2)
                nc.tensor.matmul(sp2[:, 0, :], k8_rows[:, :, ts(i, P)],
                                 q8_rows[:, :, ts(j, 512)],
                                 start=True, stop=True, perf_mode=DR)
                nc.tensor.matmul(sp2[:, 1, :], k8_rows[:, :, ts(i + 1, P)],
                                 q8_rows[:, :, ts(j, 512)],
                                 start=True, stop=True, perf_mode=DR)
                if pend is not None:
                    flush_full(pend)
                    o_unit(1)
                pend = (sp2, i)
            if pend is not None:
                flush_full(pend)

            # diagonal chunks, bf16, two chunks sharing one 2-bank PSUM
            # tile; the crossing 128-col block is masked with a causal
            # triangle via a cheap DVE multiply (keeps Pool off the chain).
            def flush_diag(pendd):
                sd, rp = pendd
                c0 = 256 * rp
                i0 = 4 * j + 2 * rp
                pt = work.tile([P, 2, 512], BF, tag="pt", bufs=2 if jidx < _kj else 6)
                nc.scalar.activation(pt[:, :, c0:], sd[:, :, c0:],
                                     mybir.ActivationFunctionType.Exp,
                                     scale=SCALE, bias=ebias_col)
                # chunk 0: mask the crossing block; chunk 1: also zero the
                # below-diagonal block the shared exp filled with exp(-2)
                nc.vector.tensor_mul(pt[:, 0, c0:c0 + P], pt[:, 0, c0:c0 + P],
                                     trimask)
                nc.vector.tensor_mul(pt[:, 1, c0:c0 + 2 * P],
                                     pt[:, 1, c0:c0 + 2 * P], trimask2)
                for q2 in range(2):
                    i = i0 + q2
                    cq = c0 + 128 * q2
                    nc.tensor.matmul(o_ps[:, cq:], v_tok[:, i, kv, :],
                                     pt[:, q2, cq:],
                                     start=(i == 0), stop=(i == nchunks - 1))

            if j == 0:
                # tile 0 runs under the phase-1 pools (overlapped with the
                # projection tail): per-chunk 1-bank PSUM, unmerged exps
                for r in range(4):
                    c0 = 128 * r
                    s_ps = psum.tile([P, 512], F32, tag="qkv", bufs=2)
                    nc.tensor.matmul(s_ps[:, c0:], k_rows[:, ts(r, P)],
                                     q_rows[:, c0:512],
                                     start=True, stop=True)
                    pt1 = work.tile([P, 512], BF, tag="pt", bufs=2 if jidx < _kj else 6)
                    nc.scalar.activation(pt1[:, c0:], s_ps[:, c0:],
                                         mybir.ActivationFunctionType.Exp,
                                         scale=SCALE, bias=ebias_col)
                    nc.vector.tensor_mul(pt1[:, c0:c0 + P], pt1[:, c0:c0 + P],
                                         trimask)
                    nc.tensor.matmul(o_ps[:, c0:], v_tok[:, r, kv, :],
                                     pt1[:, c0:],
                                     start=(r == 0), stop=(r == 3))
            else:
                pendd = None
                for rp in range(2):
                    sd = psum.tile([P, 2, 512], F32, tag="sbig", bufs=2,
                                   name="sd")
                    for q2 in range(2):
                        r = 2 * rp + q2
                        c0 = 128 * r
                        i = 4 * j + r
                        nc.tensor.matmul(sd[:, q2, c0:], k_rows[:, ts(i, P)],
                                         q_rows[:, j * 512 + c0:(j + 1) * 512],
                                         start=True, stop=True)
                    if pendd is not None:
                        flush_diag(pendd)
                    pendd = (sd, rp)
                flush_diag(pendd)
            rec1 = work.tile([1, 512], F32, tag="rec", bufs=1 if jidx < _kj else 2)
            nc.vector.reciprocal(rec1, o_ps[HD:HD + 1, :])
            rb = work.tile([64, 512], F32, tag="rb", bufs=1 if jidx < _kj else 3)
            nc.gpsimd.partition_broadcast(rb, rec1)
            dst = oT_sb[half * 64:(half + 1) * 64, mc, ts(j, 512)]
            if half == 0:
                nc.vector.tensor_mul(dst, o_ps[0:HD, :], rb)
            else:
                # DVE ops must be partition-aligned; hop through a base-0
                # temp and DMA-shift into the upper half.
                on = work.tile([64, 512], BF, tag="on", bufs=1 if jidx < _kj else 3)
                nc.vector.tensor_mul(on, o_ps[0:HD, :], rb)
                nc.sync.dma_start(out=dst, in_=on)

    # last-processed tile's output projection
    if phase == "all" and attn_tiles:
        jp = attn_tiles[-1]
        odue.extend((4 * jp + t4, nd) for t4 in range(4)
                    for nd in range(D // 512))
        if o_half[0] is not None:
            tt_h, nd_h, o_out_h = o_half[0]
            o_half[0] = None
            o_finish(tt_h, nd_h, o_out_h)
        o_unit(len(odue), alt=True)

    if taps is not None:
        for name, tl in (("qT_o", qT_sb), ("kT_o", kT_sb), ("vtok_o", v_tok),
                         ("oT_o", oT_sb)):
            if name in taps:
                nc.sync.dma_start(out=taps[name], in_=tl)
    ctx.close()


def make_aps(nc, t_len=T, ntab=2):
    return {
        "xT": nc.dram_tensor("xT", [D, 512], BF, kind="ExternalInput").ap(),
        "x8": nc.dram_tensor("x8", [D, t_len - 512], F8, kind="ExternalInput").ap(),
        "wqT": nc.dram_tensor("wqT", [D, CQ], BF, kind="ExternalInput").ap(),
        "wkT": nc.dram_tensor("wkT", [P, D // P * CKV], BF, kind="ExternalInput").ap(),
        "wvT": nc.dram_tensor("wvT", [P, D // P * CKV], BF, kind="ExternalInput").ap(),
        "wq8": nc.dram_tensor("wq8", [D, CQ], F8, kind="ExternalInput").ap(),
        "wk8": nc.dram_tensor("wk8", [P, D // P * CKV], F8, kind="ExternalInput").ap(),
        "wv8": nc.dram_tensor("wv8", [P, D // P * CKV], F8, kind="ExternalInput").ap(),
        "woT": nc.dram_tensor("woT", [CQ, D], BF, kind="ExternalInput").ap(),
        "tabs": nc.dram_tensor("tabs", [P, ntab, t_len], BF, kind="ExternalInput").ap(),
        "perm": nc.dram_tensor("perm", [P, P], BF, kind="ExternalInput").ap(),
        "ones2": nc.dram_tensor("ones2", [2, P], BF, kind="ExternalInput").ap(),
        "trimask": nc.dram_tensor("trimask", [P, P], BF, kind="ExternalInput").ap(),
        "outp": nc.dram_tensor("outp", [t_len, D], BF, kind="ExternalOutput").ap(),
    }


def build_nc(t_len=T, n_iters=1, ntab=2, num_devices=N_CORES):
    nc = bacc.Bacc("TRN2", target_bir_lowering=False, debug=False,
                   num_devices=num_devices)
    ap = make_aps(nc, t_len, ntab)
    with tile.TileContext(nc) as tc:
        for _ in range(n_iters):
            _emit_iteration(nc, tc, ap, t_len)
    nc.compile()
    return nc


# ---------------- host-side prep ----------------

def _perm64():
    """Per-head dim permutation: evens then odds."""
    p = np.concatenate([np.arange(0, HD, 2), np.arange(1, HD, 2)])
    return p


def make_tables(t_len, q_w, k_w):
    """[128, ntab, t_len] bf16: 0 = q-TCC, 1 = q-TSS (+2 = k if weights differ).

    Per 64-row head block (repeated for both heads of a 128-chunk):
      rows 0..31 (x1/even dims):  TCC = C*w_even,  TSS = -S*w_even
      rows 32..63 (x2/odd dims):  TCC = C*w_odd,   TSS = +S*w_odd
    so rope(x) = x*TCC + swap32(x)*TSS.
    """
    inv = 1.0 / (THETA ** (np.arange(0, HD, 2, dtype=np.float64) / HD))  # [32]
    t = np.arange(t_len, dtype=np.float64)
    f = t[None, :] * inv[:, None]                      # [32, t]
    c, s = np.cos(f), np.sin(f)
    shared = np.array_equal(np.asarray(q_w), np.asarray(k_w))
    pairs = ((0, q_w),) if shared else ((0, q_w), (2, k_w))
    out = np.empty((P, 2 if shared else 4, t_len), dtype=np.float32)
    for idx, w in pairs:
        w1 = np.asarray(w, np.float64)[0::2][:, None]  # even-dim weights
        w2 = np.asarray(w, np.float64)[1::2][:, None]
        cc = np.concatenate([c * w1, c * w2], axis=0)        # [64, t]
        ss = np.concatenate([-s * w1, s * w2], axis=0)       # [64, t]
        out[:, idx + 0] = np.tile(cc, (2, 1))
        out[:, idx + 1] = np.tile(ss, (2, 1))
    return out.astype(BF16)


def make_in_maps(x, Wq, Wk, Wv, Wo, q_norm_w, k_norm_w, t_len=T):
    perm = _perm64()
    tabs = make_tables(t_len, q_norm_w, k_norm_w)
    pmat = np.zeros((P, P), dtype=BF16)
    for m in range(P):
        pmat[m ^ 32, m] = 1.0   # out[m] = in[m ^ 32]
    ones2 = np.zeros((2, P), dtype=BF16)
    ones2[0, 0:64] = 1.0
    ones2[1, 64:128] = 1.0
    trimask = np.triu(np.ones((P, P), dtype=np.float32)).astype(BF16)
    xTb = [np.ascontiguousarray(x[b, :t_len].T) for b in range(B)]
    in_maps = []
    for core in range(N_CORES):
        b, g = core // GROUPS, core % GROUPS
        qr0, kr0 = g * CQ, g * CKV
        # row-permute q/k weights per head (evens then odds); 16x scale so
        # the fp8 copies clear e4m3's subnormal range (RMS cancels it for
        # q/k; the v path's 16x is divided out in the host gather)
        wq = Wq[qr0:qr0 + CQ].reshape(QH, HD, D)[:, perm].reshape(CQ, D) * WS
        wk = Wk[kr0:kr0 + CKV].reshape(KVH, HD, D)[:, perm].reshape(CKV, D) * WS
        wv = Wv[kr0:kr0 + CKV] * WS
        wo = Wo[:, qr0:qr0 + CQ]
        wqT = np.ascontiguousarray(wq.T)
        wkT = np.ascontiguousarray(wk.T)
        wvT = np.ascontiguousarray(wv.T)
        def parr(wt):
            # [D, M] "(c p) n" -> [P, c*n]: one contiguous DMA run/partition
            return np.ascontiguousarray(
                wt.reshape(D // P, P, -1).transpose(1, 0, 2).reshape(P, -1))

        def parr8(wt):
            # [D, M] "(c i p) n" -> [P, c*i*n] matching [P, DC8, 2, M] tiles
            return np.ascontiguousarray(
                wt.reshape(D // 256, 2, P, -1).transpose(2, 0, 1, 3).reshape(P, -1))

        in_maps.append({
            "xT": xTb[b][:, 0:512].astype(BF16),
            "x8": xTb[b][:, 512:].astype(E4M3),
            "wqT": wqT.astype(BF16),
            "wkT": parr(wkT.astype(BF16)),
            "wvT": parr(wvT.astype(BF16)),
            "wq8": wqT.astype(E4M3),
            "wk8": parr8(wkT.astype(E4M3)),
            "wv8": parr8(wvT.astype(E4M3)),
            "woT": np.ascontiguousarray(wo.T).astype(BF16),
            "tabs": tabs,
            "perm": pmat,
            "ones2": ones2,
            "trimask": trimask,
        })
    return in_maps


_NC_CACHE = {}


def _get_nc(ntab):
    key = ("nc", ntab)
    if key not in _NC_CACHE:
        _NC_CACHE[key] = build_nc(T, 1, ntab=ntab)
    return _NC_CACHE[key]


def kernel(x, Wq, Wk, Wv, Wo, q_norm_w, k_norm_w, mask, **_unused):
    x = np.asarray(x, np.float32)
    in_maps = make_in_maps(np.asarray(x, np.float32), np.asarray(Wq, np.float32),
                           np.asarray(Wk, np.float32), np.asarray(Wv, np.float32),
                           np.asarray(Wo, np.float32), np.asarray(q_norm_w, np.float32),
                           np.asarray(k_norm_w, np.float32))
    nc = _get_nc(in_maps[0]["tabs"].shape[1])
    res = run_bass_kernel_spmd(nc, in_maps, list(range(N_CORES)))
    out = np.zeros((B, T, D), dtype=np.float32)
    for core in range(N_CORES):
        out[core // GROUPS] += np.asarray(res.results[core]["outp"], np.float32)
    out /= WS
    return out


# revision 60
# speedup vs baseline: 1.0004x; 1.0004x over previous
"""nn_Attention_18700287607351 — GQA attention (RMSNorm + RoPE) on 8 TRN2 cores.

Sharding (per the hint): 8 shards = (batch in {0,1}) x (4 KV-head groups),
each shard owns 2 KV heads + their 8 query heads, with Wq/Wk/Wv rows and Wo
columns split by head group. Each core computes a partial [T, D] output
(row-parallel Wo); the host sums the 4 partials per batch.

Device kernel (per core, mixed bf16 / fp8-DoubleRow):
  - Precision split: the error-sensitive work — everything consumed by
    low-key-count (early) query rows — stays bf16: token-tile 0 of the
    Q/K/V projections, every diagonal 128-chunk of scores/exp/attnV, and
    the whole O-GEMM. Everything consumed only by >=512-key rows runs as
    fp8e4m3 DoubleRow matmuls (0.5 PE cycles/output column, 2x contraction
    per pass): projection tiles 1-3, off-diagonal scores, paired attnV.
    Attention averaging washes fp8 noise by ~1.65/sqrt(n_keys), so late
    rows stay ~4e-3 accurate while early rows take the bf16 path.
  - All Wq/Wk/Wv are host-scaled 16x so their fp8 copies avoid e4m3
    subnormals; RMSNorm cancels the scale for q/k, and the 16x on v rides
    through attnV + O-GEMM and is divided out in the host gather.
  - exp is computed with a folded bias of -2 (softmax-invariant) so fp8
    P' = exp(s/8 - 2) <= e^6 = 403 < 448 can never overflow e4m3.
  - Scores q8/k8 live in a (32-partition, pair) DoubleRow layout packed 4
    heads per 128 partitions (PE quadrant tile_position); the DR pair dim
    contracts dims d = 32*i + p. attnV DoubleRow instead pairs two
    128-token chunks (the pair dim = chunk parity), which is exactly the
    [128, 2, 512] layout the paired exp already produces, so one DR matmul
    consumes both chunks of a pair.
  - Otherwise the structure matches the bf16 baseline: feature-major
    qT/kT, host-permuted head dims (evens|odds) making RoPE
    partition-aligned, per-head RMS via block-ones matmuls, transposed
    scores with causal block-skipping, ones-column denominator in attnV,
    O-GEMM software-pipelined into the next tile's attention heads.
"""

import numpy as np
import ml_dtypes

import concourse.bass as bass
import concourse.bacc as bacc
import concourse.mybir as mybir
import concourse.tile as tile
from concourse.bass import ts
from concourse.bass_utils import run_bass_kernel_spmd

BF16 = ml_dtypes.bfloat16
E4M3 = ml_dtypes.float8_e4m3fn

B, T, D = 2, 2048, 2048
H, HKV, HD = 32, 8, 64
THETA = 3.0
EPS = 1e-6
SCALE = HD ** -0.5
N_CORES = 8
GROUPS = N_CORES // B          # 4 head-groups per batch
KVH = HKV // GROUPS            # 2 kv heads per core
QH = KVH * (H // HKV)          # 8 q heads per core
CQ = QH * HD                   # 512 q cols per core
CKV = KVH * HD                 # 128 kv cols per core
P = 128
WS = 16.0                      # host weight scale (fp8 subnormal avoidance)
EBIAS = -2.0                   # exp bias, softmax-invariant, fp8 overflow-proof

F32 = mybir.dt.float32
BF = mybir.dt.bfloat16
F8 = mybir.dt.float8e4
DR = mybir.MatmulPerfMode.DoubleRow


def _emit_iteration(nc, tc, ap, t_len, taps=None):
    """Emit one full per-core attention iteration into the TileContext."""
    Tl = t_len
    DC = D // P              # d chunks (16)
    DC8 = D // 256           # DoubleRow d chunk pairs (8)
    QC = CQ // P             # q col chunks (4)
    NT = Tl // 512           # 512-wide t tiles
    TC = Tl // P             # 128-wide t chunks

    import contextlib
    ctx = contextlib.ExitStack()
    const = ctx.enter_context(tc.tile_pool(name="const", bufs=1))
    projctx = contextlib.ExitStack()
    proj = projctx.enter_context(tc.tile_pool(name="proj", bufs=1))
    workp = projctx.enter_context(tc.tile_pool(name="workp", bufs=2))
    psum1 = projctx.enter_context(tc.tile_pool(name="psum1", space="PSUM", bufs=1))
    work = workp   # phase-1 alias; re-pointed to the attention pool below
    psum = psum1   # phase-1 alias

    # ---------------- persistent loads ----------------
    ntab = ap["tabs"].shape[1]
    tabs_early = const.tile([P, ntab, Tl], BF, name="tabs")
    xT_sb = proj.tile([P, DC, 512], BF)             # tokens 0..511 only (bf16 path)
    x8_sb = proj.tile([P, DC8, 2, Tl - 512], F8)    # tokens 512+ (DR path)
    wqT_sb = proj.tile([P, DC, CQ], BF)
    wkT_sb = proj.tile([P, DC, CKV], BF)
    wvT_sb = proj.tile([P, DC, CKV], BF)
    wq8_sb = proj.tile([P, DC8, 2, CQ], F8)
    wk8_sb = proj.tile([P, DC8, 2, CKV], F8)
    wv8_sb = proj.tile([P, DC8, 2, CKV], F8)
    woT_sb = const.tile([P, QC, D], BF)
    # one DMA per tensor; [(c p) n -> p c n] so each partition reads DC
    # contiguous segments. Order: K/V weights first, then x so the K-GEMM
    # accumulation can chase the loads; Wo (needed last) at the end.
    nc.sync.dma_start(out=wkT_sb, in_=ap["wkT"])
    xTview = ap["xT"].rearrange("(c p) t -> p c t", p=P)
    for c4 in range(4):
        nc.sync.dma_start(out=xT_sb[:, c4 * 4:(c4 + 1) * 4], in_=xTview[:, c4 * 4:(c4 + 1) * 4])
    nc.sync.dma_start(out=wvT_sb, in_=ap["wvT"])
    nc.sync.dma_start(out=wqT_sb, in_=ap["wqT"].rearrange("(c p) n -> p c n", p=P))
    nc.sync.dma_start(out=tabs_early, in_=ap["tabs"])
    nc.sync.dma_start(out=wk8_sb, in_=ap["wk8"])
    nc.sync.dma_start(out=wv8_sb, in_=ap["wv8"])
    nc.sync.dma_start(out=wq8_sb, in_=ap["wq8"].rearrange("(c i p) n -> p c i n", p=P, i=2))
    x8view = ap["x8"].rearrange("(c i p) t -> p c i t", p=P, i=2)
    for c2 in range(4):
        nc.sync.dma_start(out=x8_sb[:, c2 * 2:(c2 + 1) * 2], in_=x8view[:, c2 * 2:(c2 + 1) * 2])
    nc.sync.dma_start(out=woT_sb, in_=ap["woT"].rearrange("(c p) n -> p c n", p=P))
    tabs = tabs_early

    eps_col = const.tile([P, 1], F32)
    nc.vector.memset(eps_col, EPS)
    ebias_col = const.tile([P, 1], F32)
    nc.vector.memset(ebias_col, EBIAS)
    perm_sb = const.tile([P, P], BF)
    nc.sync.dma_start(out=perm_sb, in_=ap["perm"])
    trimask = const.tile([P, P], BF)
    nc.sync.dma_start(out=trimask, in_=ap["trimask"])
    trimask2 = const.tile([P, 2 * P], BF)   # [zeros | tri] for merged diag pairs
    nc.vector.memset(trimask2[:, 0:P], 0.0)
    nc.sync.dma_start(out=trimask2[:, P:2 * P], in_=ap["trimask"])
    blk128 = const.tile([P, P], BF)         # block-ones: per-head sum + broadcast
    nc.vector.memset(blk128, 0.0)
    nc.vector.memset(blk128[0:64, 0:64], 1.0)
    nc.vector.memset(blk128[64:128, 64:128], 1.0)

    # persistent activations
    q8flat = proj.tile([P, QC, Tl], F8)        # fp8 q staging (pre-shuffle)
    k8flat = proj.tile([P, Tl], F8)            # fp8 k staging (pre-shuffle)
    qT_sb = const.tile([P, QC, Tl], BF)        # feature-major q, rms+rope'd
    kT_sb = const.tile([P, Tl], BF)            # feature-major k [kv0 | kv1]
    kT_sw = const.tile([P, Tl], BF)            # swapped copy [kv1 | kv0]
    v_tok = const.tile([P, TC, KVH, HD + 1], BF)  # token-major v + ones col
    oT_sb = const.tile([P, QC, Tl], BF)        # feature-major attn out
    # fp8 DoubleRow copies: q8/k8 pack head quadrants on 32-partition bases;
    # pair dim contracts d = 32*i + p. v8 pairs two 128-token chunks.
    q8_sb = const.tile([P, 2, QH // 2, Tl], F8)   # [64b+p, i, h//2, t], b=h%2
    k8_sb = const.tile([P, 2, KVH, Tl], F8)       # kv replicated at bases {0,64}
    # dual-fp8 ldweights needs the pair stride 64-aligned: inner width 128
    v8_tok = const.tile([P, TC // 2, KVH, 2, 2 * HD], F8)

    def rms_rope(src_psum, dst, nt, tab0):
        """src_psum [128, 512] f32 (2 heads feature-major) -> dst bf16 slice.

        Row layout per head h in {0,1}: rows h*64..h*64+31 = even dims (x1),
        h*64+32..h*64+63 = odd dims (x2). RoPE = xs*TCC + swap32(xs)*TSS with
        the sign and norm_w baked into the host-built tables, so every DVE op
        is partition-aligned (walrus requires samePartitionsAll).
        """
        sq = work.tile([P, 512], BF, tag="sq", bufs=2)
        nc.scalar.activation(sq, src_psum, mybir.ActivationFunctionType.Square)
        # per-head sum over 64-partition groups, broadcast to all 64 rows
        # by the block-ones matmul itself -> [128, 512]
        SS = psum.tile([P, 512], F32, tag="ms" if int(__import__("os").environ.get("KJ0", "1")) < 2 else "sbig", bufs=2)
        nc.tensor.matmul(SS, blk128, sq, start=True, stop=True)
        rt = work.tile([P, 512], F32, tag="rr", bufs=2)
        nc.scalar.activation(rt, SS, mybir.ActivationFunctionType.Sqrt,
                             scale=1.0 / HD, bias=eps_col)
        with nc.allow_low_precision(reason="rsqrt broadcast is plenty for a 2e-2 gate"):
            nc.vector.reciprocal(rt, rt)
        xs = work.tile([P, 512], BF, tag="xs", bufs=2)
        nc.vector.tensor_mul(xs, src_psum, rt)
        # rope partner (row m ^ 32) via permutation matmul
        xw_ps = psum.tile([P, 512], F32, tag="sbig", bufs=2)
        nc.tensor.matmul(xw_ps, perm_sb, xs, start=True, stop=True)
        m1 = work.tile([P, 512], BF, tag="m1", bufs=2)
        nc.vector.tensor_mul(m1, xs, tabs[:, tab0 + 0, ts(nt, 512)])
        xw = work.tile([P, 512], BF, tag="xw", bufs=2)
        nc.scalar.copy(xw, xw_ps)
        m2 = work.tile([P, 512], BF, tag="m2", bufs=2)
        nc.vector.tensor_mul(m2, xw, tabs[:, tab0 + 1, ts(nt, 512)])
        nc.vector.tensor_add(dst, m1, m2)

    def proj_matmuls(dst_ps, wT, w8, mcols, nt):
        """Accumulate x @ W.T for token tile nt into dst_ps [128, 512].

        Tile 0 runs bf16 (early rows need it); tiles 1+ run fp8 DoubleRow.
        mcols = (col0, ncols) slice of the weight output dim.
        """
        c0, ncol = mcols
        if nt == 0:
            for c in range(DC):
                nc.tensor.matmul(dst_ps, wT[:, c, c0:c0 + ncol],
                                 xT_sb[:, c, :],
                                 start=(c == 0), stop=(c == DC - 1))
        else:
            for c in range(DC8):
                nc.tensor.matmul(dst_ps, w8[:, c, :, c0:c0 + ncol],
                                 x8_sb[:, c, :, ts(nt - 1, 512)],
                                 start=(c == 0), stop=(c == DC8 - 1),
                                 perf_mode=DR)

    def to_fp8(src_bf, dst_f8):
        """bf16 -> fp8 dtype conversion on the (mostly idle) Pool engine."""
        nc.gpsimd.tensor_copy(out=dst_f8, in_=src_bf)

    # ---------------- K / V / Q, software-pipelined ----------------
    # Emit unit u+1's GEMM chain BEFORE unit u's rms/post ops so the
    # in-order PE queue never stalls head-of-line behind ACT/DVE work;
    # continuous PE bursts also keep the tensor engine at full p-state.
    def post_K(ps, nt):
        rms_rope(ps, kT_sb[:, ts(nt, 512)], nt, ntab - 2)
        # swapped copy so every q-head finds its kv head at its own base
        # partition (matmul requires lhsT/rhs partition bases to match)
        nc.sync.dma_start(out=kT_sw[0:64, ts(nt, 512)], in_=kT_sb[64:128, ts(nt, 512)])
        nc.sync.dma_start(out=kT_sw[64:128, ts(nt, 512)], in_=kT_sb[0:64, ts(nt, 512)])
        to_fp8(kT_sb[:, ts(nt, 512)], k8flat[:, ts(nt, 512)])

    def post_V(ps, nt):
        vT_tmp = work.tile([P, 512], BF, tag="vT", bufs=2)
        nc.scalar.copy(vT_tmp, ps)
        for i4 in range(4):
            tchunk = nt * 4 + i4
            # XBAR transpose needs a contiguous destination; stage then
            # copy into the strided [.., kv, 0:64] layout on gpsimd.
            vstg = work.tile([P, P], BF, tag="vstg")
            nc.sync.dma_start(out=vstg, in_=vT_tmp[:, ts(i4, P)], transpose=True)
            nc.gpsimd.tensor_copy(
                out=v_tok[:, tchunk, :, 0:HD],
                in_=vstg.rearrange("p (h c) -> p h c", h=KVH))
            nc.gpsimd.tensor_copy(
                out=v8_tok[:, tchunk // 2, :, tchunk % 2, 0:HD],
                in_=vstg.rearrange("p (h c) -> p h c", h=KVH))

    def post_Q(ps, nt, mc):
        rms_rope(ps, qT_sb[:, mc, ts(nt, 512)], nt, 0)
        if nt > 0:
            # fp8 copies only feed off-diagonal scores (query tiles 1+)
            to_fp8(qT_sb[:, mc, ts(nt, 512)], q8flat[:, mc, ts(nt, 512)])

    units = []
    for nt in range(NT):
        units.append(("K", nt, None))
        units.append(("V", nt, None))
        units.extend(("Q", nt, mc) for mc in range(QC))
    inflight = []

    def drain_one():
        pkind, pnt, pmc, pps = inflight.pop(0)
        if pkind == "K":
            post_K(pps, pnt)
        elif pkind == "V":
            post_V(pps, pnt)
        else:
            post_Q(pps, pnt, pmc)

    for uidx, (kind, nt, mc) in enumerate(units):
        ps = psum.tile([P, 512], F32, tag="opsum" if uidx % 2 == 0 else "qkv",
                       bufs=2, name="ps")
        if kind == "K":
            proj_matmuls(ps, wkT_sb, wk8_sb, (0, CKV), nt)
        elif kind == "V":
            proj_matmuls(ps, wvT_sb, wv8_sb, (0, CKV), nt)
        else:
            proj_matmuls(ps, wqT_sb, wq8_sb, (mc * P, P), nt)
        inflight.append((kind, nt, mc, ps))
        import os as _os
        if len(inflight) > int(_os.environ.get('KRA', '1')):
            drain_one()
    while inflight:
        drain_one()
    nc.vector.memset(v_tok[:, :, :, HD], 1.0)
    nc.vector.memset(v8_tok[:, :, :, :, HD], 1.0)


    # ---------------- attention + output projection, tile-major ----------------
    # j outer / heads inner so the O-GEMM for tile j's rows can start right
    # after its 8 heads finish, filling PE idle under the ACT-bound exp phase.
    import os
    phase = os.environ.get("KPHASE", "all")
    attn_tiles = list(range(NT)) if phase in ("all", "attn") else []

    odue = []

    ob_cur = [None]
    o_alt = [0]
    o_half = [None]

    def o_finish(tt, nd, o_out):
        for mc in (2, 3):
            nc.tensor.matmul(o_out, oT_sb[:, mc, ts(tt, P)],
                             woT_sb[:, mc, ts(nd, 512)],
                             start=False, stop=(mc == QC - 1))
        if nd == 0:
            ob_cur[0] = work.tile([P, D], BF, tag="ob", bufs=4, name="ob")
        nc.vector.tensor_copy(ob_cur[0][:, ts(nd, 512)], o_out)
        if nd == D // 512 - 1:
            nc.sync.dma_start(out=ap["outp"][ts(tt, P), :], in_=ob_cur[0])

    def o_unit(nmax=1, alt=False):
        # one [128-token, 512-outcol] O-GEMM unit: 4 PE matmuls + DVE copy;
        # the 4 units of a token chunk share one ob tile and one output DMA.
        # Mid-tile drips emit HALF units (2 matmuls) so the PE-queue detour
        # between score pairs stays under ACT's per-pair exp time; the
        # drain (alt=True) runs full units.
        for _ in range(nmax):
            if o_half[0] is not None:
                tt, nd, o_out = o_half[0]
                o_half[0] = None
                o_finish(tt, nd, o_out)
                continue
            if not odue:
                return
            tt, nd = odue.pop(0)
            o_alt[0] += 1
            tag = "qkv" if not alt or o_alt[0] % 2 else "opsum"
            o_out = psum.tile([P, 512], F32, tag=tag, bufs=2, name="o_out")
            for mc in (0, 1):
                nc.tensor.matmul(o_out, oT_sb[:, mc, ts(tt, P)],
                                 woT_sb[:, mc, ts(nd, 512)],
                                 start=(mc == 0), stop=False)
            if alt:
                o_finish(tt, nd, o_out)
            else:
                o_half[0] = (tt, nd, o_out)

    import os as _os2
    _kj = int(_os2.environ.get("KJ0", "1"))
    for jidx, j in enumerate(attn_tiles):
        if jidx == 1:
            # fp8 DoubleRow shuffles: one wide partition-shift DMA per
            # (head, i) — batched here so no waiting DMA ever blocks the
            # in-order SP queue mid-phase
            for kv in range(KVH):
                for i in range(2):
                    srcv = k8flat[64 * kv + 32 * i:64 * kv + 32 * i + 32, :]
                    for b in range(2):
                        nc.sync.dma_start(out=k8_sb[64 * b:64 * b + 32, i, kv, :],
                                          in_=srcv)
            for h8 in range(QH):
                mc8, half8 = h8 // 2, h8 % 2
                for i in range(2):
                    nc.sync.dma_start(
                        out=q8_sb[64 * half8:64 * half8 + 32, i, h8 // 2, 512:],
                        in_=q8flat[64 * half8 + 32 * i:64 * half8 + 32 * i + 32,
                                   mc8, 512:])
        if jidx == _kj:
            # release projection inputs + phase-1 work/psum; later tiles use
            # the attention pools (earlier tiles ran on phase-1 pools,
            # overlapped under the projection tail)
            projctx.close()
            work = ctx.enter_context(tc.tile_pool(name="worka", bufs=2))
            psum = ctx.enter_context(tc.tile_pool(name="psum2", space="PSUM", bufs=1))
        if phase == "all" and jidx > 0:
            jp = attn_tiles[jidx - 1]
            odue.extend((4 * jp + t4, nd) for t4 in range(4)
                        for nd in range(D // 512))
        for h in range(QH):
            # previous tile's output projection units drip in only after
            # each pair flush — never at a head/tile boundary, where the
            # next head's scores must reach ACT without a PE detour
            kv = h // (QH // KVH)
            mc, half = h // 2, h % 2
            q_rows = qT_sb[half * 64:(half + 1) * 64, mc, :]
            ksrc = kT_sb if kv == half else kT_sw
            k_rows = ksrc[half * 64:(half + 1) * 64, :]
            b64 = 64 * (h % 2)
            q8_rows = q8_sb[b64:b64 + 32, :, h // 2, :]
            k8_rows = k8_sb[b64:b64 + 32, :, kv, :]
            o_ps = psum.tile([HD + 1, 512], F32, tag="opsum", bufs=2)
            nchunks = 4 * (j + 1)

            # full (below-diagonal) chunks in PAIRS: fp8 DoubleRow scores
            # (2x), one exp instruction over [128, 2, 512] writing fp8
            # (amortizes the ACT PSUM access penalty), then ONE DoubleRow
            # attnV matmul consuming both chunks (pair dim = chunk parity).
            # Scores run one pair AHEAD of exp/attnV so the PE queue never
            # stalls head-of-line behind an exp it just fed.
            def flush_full(pend):
                sp2, i = pend
                pt8 = work.tile([P, 2, 512], F8, tag="pt8", bufs=2 if jidx < _kj else 8)
                nc.scalar.activation(pt8, sp2, mybir.ActivationFunctionType.Exp,
                                     scale=SCALE, bias=ebias_col)
                nc.tensor.matmul(o_ps, v8_tok[:, i // 2, kv, :, 0:HD + 1], pt8,
                                 start=(i == 0), stop=False, perf_mode=DR)

            pend = None
            for i in range(0, 4 * j, 2):
                sp2 = psum.tile([P, 2, 512], F32, tag="sbig", bufs=2)
                nc.tensor.matmul(sp2[:, 0, :], k8_rows[:, :, ts(i, P)],
                                 q8_rows[:, :, ts(j, 512)],
                                 start=True, stop=True, perf_mode=DR)
                nc.tensor.matmul(sp2[:, 1, :], k8_rows[:, :, ts(i + 1, P)],
                                 q8_rows[:, :, ts(j, 512)],
                                 start=True, stop=True, perf_mode=DR)
                if pend is not None:
                    flush_full(pend)
                    o_unit(1)
                pend = (sp2, i)
            if pend is not None:
                flush_full(pend)

            # diagonal chunks, bf16, two chunks sharing one 2-bank PSUM
            # tile; the crossing 128-col block is masked with a causal
            # triangle via a cheap DVE multiply (keeps Pool off the chain).
            def flush_diag(pendd):
                sd, rp = pendd
                c0 = 256 * rp
                i0 = 4 * j + 2 * rp
                pt = work.tile([P, 2, 512], BF, tag="pt", bufs=2 if jidx < _kj else 6)
                nc.scalar.activation(pt[:, :, c0:], sd[:, :, c0:],
                                     mybir.ActivationFunctionType.Exp,
                                     scale=SCALE, bias=ebias_col)
                # chunk 0: mask the crossing block; chunk 1: also zero the
                # below-diagonal block the shared exp filled with exp(-2)
                nc.vector.tensor_mul(pt[:, 0, c0:c0 + P], pt[:, 0, c0:c0 + P],
                                     trimask)
                nc.vector.tensor_mul(pt[:, 1, c0:c0 + 2 * P],
                                     pt[:, 1, c0:c0 + 2 * P], trimask2)
                for q2 in range(2):
                    i = i0 + q2
                    cq = c0 + 128 * q2
                    nc.tensor.matmul(o_ps[:, cq:], v_tok[:, i, kv, :],
                                     pt[:, q2, cq:],
                                     start=(i == 0), stop=(i == nchunks - 1))

            if j == 0:
                # tile 0 runs under the phase-1 pools (overlapped with the
                # projection tail): per-chunk 1-bank PSUM, unmerged exps
                for r in range(4):
                    c0 = 128 * r
                    s_ps = psum.tile([P, 512], F32, tag="qkv", bufs=2)
                    nc.tensor.matmul(s_ps[:, c0:], k_rows[:, ts(r, P)],
                                     q_rows[:, c0:512],
                                     start=True, stop=True)
                    pt1 = work.tile([P, 512], BF, tag="pt", bufs=2 if jidx < _kj else 6)
                    nc.scalar.activation(pt1[:, c0:], s_ps[:, c0:],
                                         mybir.ActivationFunctionType.Exp,
                                         scale=SCALE, bias=ebias_col)
                    nc.vector.tensor_mul(pt1[:, c0:c0 + P], pt1[:, c0:c0 + P],
                                         trimask)
                    nc.tensor.matmul(o_ps[:, c0:], v_tok[:, r, kv, :],
                                     pt1[:, c0:],
                                     start=(r == 0), stop=(r == 3))
            else:
                pendd = None
                for rp in range(2):
                    sd = psum.tile([P, 2, 512], F32, tag="sbig", bufs=2,
                                   name="sd")
                    for q2 in range(2):
                        r = 2 * rp + q2
                        c0 = 128 * r
                        i = 4 * j + r
                        nc.tensor.matmul(sd[:, q2, c0:], k_rows[:, ts(i, P)],
                                         q_rows[:, j * 512 + c0:(j + 1) * 512],
                                         start=True, stop=True)
                    if pendd is not None:
                        flush_diag(pendd)
                    pendd = (sd, rp)
                flush_diag(pendd)
            rec1 = work.tile([1, 512], F32, tag="rec", bufs=1 if jidx < _kj else 2)
            nc.vector.reciprocal(rec1, o_ps[HD:HD + 1, :])
            rb = work.tile([64, 512], F32, tag="rb", bufs=1 if jidx < _kj else 3)
            nc.gpsimd.partition_broadcast(rb, rec1)
            dst = oT_sb[half * 64:(half + 1) * 64, mc, ts(j, 512)]
            if half == 0:
                nc.vector.tensor_mul(dst, o_ps[0:HD, :], rb)
            else:
                # DVE ops must be partition-aligned; hop through a base-0
                # temp and DMA-shift into the upper half.
                on = work.tile([64, 512], BF, tag="on", bufs=1 if jidx < _kj else 3)
                nc.vector.tensor_mul(on, o_ps[0:HD, :], rb)
                nc.sync.dma_start(out=dst, in_=on)

    # last-processed tile's output projection
    if phase == "all" and attn_tiles:
        jp = attn_tiles[-1]
        odue.extend((4 * jp + t4, nd) for t4 in range(4)
                    for nd in range(D // 512))
        if o_half[0] is not None:
            tt_h, nd_h, o_out_h = o_half[0]
            o_half[0] = None
            o_finish(tt_h, nd_h, o_out_h)
        o_unit(len(odue), alt=True)

    if taps is not None:
        for name, tl in (("qT_o", qT_sb), ("kT_o", kT_sb), ("vtok_o", v_tok),
                         ("oT_o", oT_sb)):
            if name in taps:
                nc.sync.dma_start(out=taps[name], in_=tl)
    ctx.close()


def make_aps(nc, t_len=T, ntab=2):
    return {
        "xT": nc.dram_tensor("xT", [D, 512], BF, kind="ExternalInput").ap(),
        "x8": nc.dram_tensor("x8", [D, t_len - 512], F8, kind="ExternalInput").ap(),
        "wqT": nc.dram_tensor("wqT", [D, CQ], BF, kind="ExternalInput").ap(),
        "wkT": nc.dram_tensor("wkT", [P, D // P * CKV], BF, kind="ExternalInput").ap(),
        "wvT": nc.dram_tensor("wvT", [P, D // P * CKV], BF, kind="ExternalInput").ap(),
        "wq8": nc.dram_tensor("wq8", [D, CQ], F8, kind="ExternalInput").ap(),
        "wk8": nc.dram_tensor("wk8", [P, D // P * CKV], F8, kind="ExternalInput").ap(),
        "wv8": nc.dram_tensor("wv8", [P, D // P * CKV], F8, kind="ExternalInput").ap(),
        "woT": nc.dram_tensor("woT", [CQ, D], BF, kind="ExternalInput").ap(),
        "tabs": nc.dram_tensor("tabs", [P, ntab, t_len], BF, kind="ExternalInput").ap(),
        "perm": nc.dram_tensor("perm", [P, P], BF, kind="ExternalInput").ap(),
        "ones2": nc.dram_tensor("ones2", [2, P], BF, kind="ExternalInput").ap(),
        "trimask": nc.dram_tensor("trimask", [P, P], BF, kind="ExternalInput").ap(),
        "outp": nc.dram_tensor("outp", [t_len, D], BF, kind="ExternalOutput").ap(),
    }


def build_nc(t_len=T, n_iters=1, ntab=2, num_devices=N_CORES):
    nc = bacc.Bacc("TRN2", target_bir_lowering=False, debug=False,
                   num_devices=num_devices)
    ap = make_aps(nc, t_len, ntab)
    with tile.TileContext(nc) as tc:
        for _ in range(n_iters):
            _emit_iteration(nc, tc, ap, t_len)
    nc.compile()
    return nc


# ---------------- host-side prep ----------------

def _perm64():
    """Per-head dim permutation: evens then odds."""
    p = np.concatenate([np.arange(0, HD, 2), np.arange(1, HD, 2)])
    return p


def make_tables(t_len, q_w, k_w):
    """[128, ntab, t_len] bf16: 0 = q-TCC, 1 = q-TSS (+2 = k if weights differ).

    Per 64-row head block (repeated for both heads of a 128-chunk):
      rows 0..31 (x1/even dims):  TCC = C*w_even,  TSS = -S*w_even
      rows 32..63 (x2/odd dims):  TCC = C*w_odd,   TSS = +S*w_odd
    so rope(x) = x*TCC + swap32(x)*TSS.
    """
    inv = 1.0 / (THETA ** (np.arange(0, HD, 2, dtype=np.float64) / HD))  # [32]
    t = np.arange(t_len, dtype=np.float64)
    f = t[None, :] * inv[:, None]                      # [32, t]
    c, s = np.cos(f), np.sin(f)
    shared = np.array_equal(np.asarray(q_w), np.asarray(k_w))
    pairs = ((0, q_w),) if shared else ((0, q_w), (2, k_w))
    out = np.empty((P, 2 if shared else 4, t_len), dtype=np.float32)
    for idx, w in pairs:
        w1 = np.asarray(w, np.float64)[0::2][:, None]  # even-dim weights
        w2 = np.asarray(w, np.float64)[1::2][:, None]
        cc = np.concatenate([c * w1, c * w2], axis=0)        # [64, t]
        ss = np.concatenate([-s * w1, s * w2], axis=0)       # [64, t]
        out[:, idx + 0] = np.tile(cc, (2, 1))
        out[:, idx + 1] = np.tile(ss, (2, 1))
    return out.astype(BF16)


def make_in_maps(x, Wq, Wk, Wv, Wo, q_norm_w, k_norm_w, t_len=T):
    perm = _perm64()
    tabs = make_tables(t_len, q_norm_w, k_norm_w)
    pmat = np.zeros((P, P), dtype=BF16)
    for m in range(P):
        pmat[m ^ 32, m] = 1.0   # out[m] = in[m ^ 32]
    ones2 = np.zeros((2, P), dtype=BF16)
    ones2[0, 0:64] = 1.0
    ones2[1, 64:128] = 1.0
    trimask = np.triu(np.ones((P, P), dtype=np.float32)).astype(BF16)
    xTb = [np.ascontiguousarray(x[b, :t_len].T) for b in range(B)]
    in_maps = []
    for core in range(N_CORES):
        b, g = core // GROUPS, core % GROUPS
        qr0, kr0 = g * CQ, g * CKV
        # row-permute q/k weights per head (evens then odds); 16x scale so
        # the fp8 copies clear e4m3's subnormal range (RMS cancels it for
        # q/k; the v path's 16x is divided out in the host gather)
        wq = Wq[qr0:qr0 + CQ].reshape(QH, HD, D)[:, perm].reshape(CQ, D) * WS
        wk = Wk[kr0:kr0 + CKV].reshape(KVH, HD, D)[:, perm].reshape(CKV, D) * WS
        wv = Wv[kr0:kr0 + CKV] * WS
        wo = Wo[:, qr0:qr0 + CQ]
        wqT = np.ascontiguousarray(wq.T)
        wkT = np.ascontiguousarray(wk.T)
        wvT = np.ascontiguousarray(wv.T)
        def parr(wt):
            # [D, M] "(c p) n" -> [P, c*n]: one contiguous DMA run/partition
            return np.ascontiguousarray(
                wt.reshape(D // P, P, -1).transpose(1, 0, 2).reshape(P, -1))

        def parr8(wt):
            # [D, M] "(c i p) n" -> [P, c*i*n] matching [P, DC8, 2, M] tiles
            return np.ascontiguousarray(
                wt.reshape(D // 256, 2, P, -1).transpose(2, 0, 1, 3).reshape(P, -1))

        in_maps.append({
            "xT": xTb[b][:, 0:512].astype(BF16),
            "x8": xTb[b][:, 512:].astype(E4M3),
            "wqT": wqT.astype(BF16),
            "wkT": parr(wkT.astype(BF16)),
            "wvT": parr(wvT.astype(BF16)),
            "wq8": wqT.astype(E4M3),
            "wk8": parr8(wkT.astype(E4M3)),
            "wv8": parr8(wvT.astype(E4M3)),
            "woT": np.ascontiguousarray(wo.T).astype(BF16),
            "tabs": tabs,
            "perm": pmat,
            "ones2": ones2,
            "trimask": trimask,
        })
    return in_maps


_NC_CACHE = {}


def _get_nc(ntab):
    key = ("nc", ntab)
    if key not in _NC_CACHE:
        _NC_CACHE[key] = build_nc(T, 1, ntab=ntab)
    return _NC_CACHE[key]


def kernel(x, Wq, Wk, Wv, Wo, q_norm_w, k_norm_w, mask, **_unused):
    x = np.asarray(x, np.float32)
    in_maps = make_in_maps(np.asarray(x, np.float32), np.asarray(Wq, np.float32),
                           np.asarray(Wk, np.float32), np.asarray(Wv, np.float32),
                           np.asarray(Wo, np.float32), np.asarray(q_norm_w, np.float32),
                           np.asarray(k_norm_w, np.float32))
    nc = _get_nc(in_maps[0]["tabs"].shape[1])
    res = run_bass_kernel_spmd(nc, in_maps, list(range(N_CORES)))
    out = np.zeros((B, T, D), dtype=np.float32)
    for core in range(N_CORES):
        out[core // GROUPS] += np.asarray(res.results[core]["outp"], np.float32)
    out /= WS
    return out
